# revision 1
# baseline (speedup 1.0000x reference)
"""Trainium2 Bass kernel for a pre-norm transformer block (MHA + MLP).

Sharding: sequence-parallel over 8 cores. Each core owns 512 tokens
(batch b = core//4, token block core%4). All weights are replicated.
The only collectives are two 4-rank AllGathers (K^T and V) inside each
batch group, replacing Megatron-style AllReduces (4 MB vs 16 MB payload).

Dataflow is feature-major (channels on partitions, tokens on the free
axis) end-to-end, so no on-chip transposes are needed:
  - LN mean/var via ones-matmul partition reduction on the TensorEngine
  - scores S^T[k, q] per head with softmax over the partition (k) axis:
    exp is fused into the PSUM->SBUF move on the ScalarEngine, and the
    softmax denominator comes free from an appended ones-column in V
  - odd heads run as base-64 quadrant matmuls (PE tile_position)
  - matmuls run in float32r (11-bit mantissa, 1 cycle/row) with weights
    pre-rounded on the host; the residual path stays exact fp32
"""
import sys

sys.path.insert(0, "/opt/trn_rl_repo")
import numpy as np
import concourse.bass as bass
import concourse.mybir as mybir
import concourse.tile as tile
from concourse import bacc
from concourse.bass_utils import run_bass_kernel_spmd

# problem shapes (hardcoded per contract)
B, N, D = 2, 2048, 1024
H, DH = 16, 64
HID = 4096
NCORES = 8
TOK = (B * N) // NCORES  # 512 tokens per core
EPS = 1e-5
SCALE = DH**-0.5
P = 128
CH = D // P  # 8 channel chunks of the model dim
KC = N // P  # 16 key chunks of the full sequence
HCH = HID // P  # 32 hidden chunks
RANKS = 4  # per-batch replica group size

F32 = mybir.dt.float32
F32R = mybir.dt.float32r
AF = mybir.ActivationFunctionType
OP = mybir.AluOpType

REPLICA_GROUPS = [[0, 1, 2, 3], [4, 5, 6, 7]]


def round_fp32r(x: np.ndarray) -> np.ndarray:
    """Round fp32 to fp32r (8-bit exp, 11-bit mantissa, RNE) on host."""
    u = np.ascontiguousarray(x, dtype=np.float32).view(np.uint32)
    u = (u + 0x7FF + ((u >> 12) & 1)) & np.uint32(0xFFFFF000)
    return u.view(np.float32)


def _ln_stripe(v: np.ndarray) -> np.ndarray:
    """[D] per-channel vector -> [P, D//P] feature-major stripe (c = ch*128+p)."""
    return np.ascontiguousarray(np.asarray(v).reshape(-1, P).T.astype(np.float32))


ALL_STAGES = ("ln1", "qkv", "cc", "attn", "exp", "proj", "ln2", "mlp")


def build_program(stages=None, do_compile=True):
    if stages is None:
        stages = set(ALL_STAGES)
    stages = set(stages)
    nc = bacc.Bacc("TRN2", target_bir_lowering=False, debug=False, num_devices=NCORES)

    # ---- kernel I/O ----
    xT = nc.dram_tensor("xT", [D, TOK], F32, kind="ExternalInput").ap()
    qkv_wT = nc.dram_tensor("qkv_wT", [D, 3 * D], F32R, kind="ExternalInput").ap()
    proj_wT = nc.dram_tensor("proj_wT", [D, D], F32R, kind="ExternalInput").ap()
    fc1_wT = nc.dram_tensor("fc1_wT", [D, HID], F32R, kind="ExternalInput").ap()
    fc2_wT = nc.dram_tensor("fc2_wT", [HID, D], F32R, kind="ExternalInput").ap()
    ln1g = nc.dram_tensor("ln1g", [P, CH], F32, kind="ExternalInput").ap()
    ln1b = nc.dram_tensor("ln1b", [P, CH], F32, kind="ExternalInput").ap()
    ln2g = nc.dram_tensor("ln2g", [P, CH], F32, kind="ExternalInput").ap()
    ln2b = nc.dram_tensor("ln2b", [P, CH], F32, kind="ExternalInput").ap()
    projb = nc.dram_tensor("projb", [P, CH], F32, kind="ExternalInput").ap()
    fc1b = nc.dram_tensor("fc1b", [P, HCH], F32, kind="ExternalInput").ap()
    fc1mg = nc.dram_tensor("fc1mg", [P, HCH], F32, kind="ExternalInput").ap()
    fc2b = nc.dram_tensor("fc2b", [P, CH], F32, kind="ExternalInput").ap()
    outT = nc.dram_tensor("outT", [D, TOK], F32, kind="ExternalOutput").ap()

    xT_chunks = xT.rearrange("(ch p) t -> p ch t", p=P)

    with tile.TileContext(nc) as tc:
        with (
            tc.tile_pool(name="consts", bufs=1) as consts,
            tc.tile_pool(name="bigs", bufs=1) as bigs,
            tc.tile_pool(name="work", bufs=3) as work,
            tc.tile_pool(name="wpool", bufs=5) as wpool,
            tc.tile_pool(name="kv", bufs=2) as kvpool,
            tc.tile_pool(name="pp", bufs=2) as ppool,
            tc.tile_pool(name="rows", bufs=3) as rows,
            tc.tile_pool(name="bc", bufs=2) as bcpool,
            tc.tile_pool(name="stg", bufs=3) as stg,
            tc.tile_pool(name="dram", bufs=1, space="DRAM") as dram,
        ):
            # ---- constants ----
            ones_r = consts.tile([P, 1], F32R)
            nc.vector.memset(ones_r[:].bitcast(F32), 1.0)
            eps_row = consts.tile([1, 1], F32, tag="eps")
            nc.vector.memset(eps_row[:], EPS)
            ln1g_sb = consts.tile([P, CH], F32, tag="ln1g")
            ln1b_sb = consts.tile([P, CH], F32, tag="ln1b")
            ln2g_sb = consts.tile([P, CH], F32, tag="ln2g")
            ln2b_sb = consts.tile([P, CH], F32, tag="ln2b")
            projb_sb = consts.tile([P, CH], F32, tag="projb")
            fc1b_sb = consts.tile([P, HCH], F32, tag="fc1b")
            fc1mg_sb = consts.tile([P, HCH], F32, tag="fc1mg")
            fc2b_sb = consts.tile([P, CH], F32, tag="fc2b")

            # ---- collective DRAM buffers ----
            kv_in0 = dram.tile([D, TOK], F32R, tag="kvin0")
            kv_in1 = dram.tile([D, TOK], F32R, tag="kvin1")
            kv_out0 = dram.tile([RANKS * D, TOK], F32R, tag="kvout0")
            kv_out1 = dram.tile([RANKS * D, TOK], F32R, tag="kvout1")
            kv_ins = (kv_in0, kv_in1)
            kv_outs = (kv_out0, kv_out1)

            def layer_norm(get_chunk, g_sb, b_sb, dst, ps_row, xr_dst=None):
                psum_mu = ps_row.tile([1, TOK], F32, tag="row")
                psum_s2 = ps_row.tile([1, TOK], F32, tag="row")
                for ch in range(CH):
                    src = get_chunk(ch)
                    if xr_dst is None:
                        xrt = work.tile([P, TOK], F32R, tag="xr", name=f"xr_{ch}")
                        xr = xrt[:]
                    else:
                        xr = xr_dst[:, ch, :]
                    nc.gpsimd.tensor_copy(out=xr, in_=src)
                    nc.tensor.matmul(
                        psum_mu[:],
                        ones_r[:],
                        xr,
                        start=(ch == 0),
                        stop=(ch == CH - 1),
                    )
                    sq = work.tile([P, TOK], F32R, tag="sq")
                    nc.vector.tensor_mul(out=sq[:], in0=src, in1=src)
                    nc.tensor.matmul(
                        psum_s2[:],
                        ones_r[:],
                        sq[:],
                        start=(ch == 0),
                        stop=(ch == CH - 1),
                    )
                mu = rows.tile([1, TOK], F32, tag="r")
                nc.vector.tensor_scalar_mul(mu[:], psum_mu[:], 1.0 / D)
                var = rows.tile([1, TOK], F32, tag="r")
                nc.vector.tensor_tensor(var[:], mu[:], mu[:], OP.mult)
                ex2 = rows.tile([1, TOK], F32, tag="r")
                nc.vector.tensor_scalar_mul(ex2[:], psum_s2[:], 1.0 / D)
                nc.vector.tensor_sub(var[:], ex2[:], var[:])
                rstd = rows.tile([1, TOK], F32, tag="r")
                nc.scalar.activation(
                    out=rstd[:], in_=var[:], func=AF.Sqrt, bias=eps_row[:]
                )
                nc.vector.reciprocal(rstd[:], rstd[:])
                cpos = rows.tile([1, TOK], F32, tag="r")
                nc.vector.tensor_tensor(cpos[:], mu[:], rstd[:], OP.mult)
                rstd_b = bcpool.tile([P, TOK], F32, tag="bc")
                nc.gpsimd.partition_broadcast(rstd_b[:], rstd[:])
                c_b = bcpool.tile([P, TOK], F32, tag="bc")
                nc.gpsimd.partition_broadcast(c_b[:], cpos[:])
                if dst is None:
                    return rstd_b, c_b
                for ch in range(CH):
                    src = get_chunk(ch)
                    t1 = work.tile([P, TOK], F32, tag="t1")
                    nc.vector.tensor_mul(t1[:], src, rstd_b[:])
                    nc.vector.tensor_sub(t1[:], t1[:], c_b[:])
                    nc.scalar.activation(
                        out=dst[:, ch, :],
                        in_=t1[:],
                        func=AF.Identity,
                        bias=b_sb[:, ch : ch + 1],
                        scale=g_sb[:, ch : ch + 1],
                    )

            # ---- stage 1: LN1 (x streamed from DRAM) ----
            for t_, s_ in (
                (ln1g_sb, ln1g),
                (ln1b_sb, ln1b),
                (ln2g_sb, ln2g),
                (ln2b_sb, ln2b),
                (projb_sb, projb),
                (fc1b_sb, fc1b),
                (fc1mg_sb, fc1mg),
                (fc2b_sb, fc2b),
            ):
                nc.gpsimd.dma_start(t_[:], s_[:])
            h1 = bigs.tile([P, CH, TOK], F32R, tag="h12")

            def ln1_chunk(ch):
                xc = work.tile([P, TOK], F32, tag="xc")
                nc.sync.dma_start(xc[:], xT_chunks[:, ch, :])
                return xc[:]

            if "ln1" in stages:
                with tc.tile_pool(name="ps_row1", bufs=2, space="PSUM") as prow:
                    layer_norm(ln1_chunk, ln1g_sb, ln1b_sb, h1, prow)

            # ---- stage 2: QKV (K first so its AllGather overlaps Q/V) ----
            st2_pool_cm = tc.tile_pool(name="ps_mm2", bufs=4, space="PSUM")
            ps_acc = st2_pool_cm.__enter__()

            # K^T tiles -> DRAM collective input (ch-outer over m-groups)
            def qkv_mgroup(ms, col0, consume):
                ws, psums = [], []
                for m in ms:
                    w = wpool.tile([P, CH, P], F32R, tag="wcol8")
                    nc.sync.dma_start(
                        w[:],
                        qkv_wT[:, col0 + m * P : col0 + (m + 1) * P].rearrange(
                            "(ch p) o -> p ch o", p=P
                        ),
                    )
                    ws.append(w)
                    psums.append(ps_acc.tile([P, TOK], F32, tag="acc", name=f"ps_{m}"))
                for ch in range(CH):
                    for i, m in enumerate(ms):
                        nc.tensor.matmul(
                            psums[i][:],
                            ws[i][:, ch, :],
                            h1[:, ch, :],
                            start=(ch == 0),
                            stop=(ch == CH - 1),
                        )
                for i, m in enumerate(ms):
                    consume(m, psums[i])

            def k_consume(m, psum):
                ktmp = stg.tile([P, TOK], F32R, tag="cp")
                nc.vector.tensor_copy(out=ktmp[:], in_=psum[:])
                nc.sync.dma_start(
                    kv_ins[m // 4][(m % 4) * P : (m % 4 + 1) * P, :], ktmp[:]
                )

            if "qkv" in stages:
                for g0 in range(4):
                    qkv_mgroup(range(g0, g0 + 1), D, k_consume)

            # V token-major [t, vout]: lhsT = h1 chunk, rhs = W_v columns
            def v_phase(vh):
                wvh = []
                for chh in range(2):
                    wv4 = wpool.tile(
                        [P, CH // 2, 512], F32R, tag="wv", bufs=2,
                        name=f"wv_{vh}_{chh}",
                    )
                    nc.sync.dma_start(
                        wv4[:],
                        qkv_wT[
                            chh * 512 : (chh + 1) * 512,
                            2 * D + vh * 512 : 2 * D + (vh + 1) * 512,
                        ].rearrange("(ch p) o -> p ch o", p=P),
                    )
                    wvh.append(wv4)
                for tt in range(TOK // P):
                    psum = ps_acc.tile([P, TOK], F32, tag="acc", name=f"psv{vh}_{tt}")
                    for ch in range(CH):
                        nc.tensor.matmul(
                            psum[:],
                            h1[:, ch, tt * P : (tt + 1) * P],
                            wvh[ch // 4][:, ch % 4, :],
                            start=(ch == 0),
                            stop=(ch == CH - 1),
                        )
                    vtmp = stg.tile([P, TOK], F32R, tag="cp", name=f"vtmp{vh}_{tt}")
                    nc.vector.tensor_copy(out=vtmp[:], in_=psum[:])
                    nc.sync.dma_start(
                        kv_ins[vh][TOK + tt * P : TOK + (tt + 1) * P, :],
                        vtmp[:],
                    )
                if "cc" in stages:
                    nc.gpsimd.collective_compute(
                        "AllGather",
                        OP.bypass,
                        ins=[kv_ins[vh][:].opt()],
                        outs=[kv_outs[vh][:].opt()],
                        replica_groups=REPLICA_GROUPS,
                    )

            if "qkv" in stages:
                v_phase(0)
                for g0 in range(4, CH):
                    qkv_mgroup(range(g0, g0 + 1), D, k_consume)
            # Q^T tiles stay in SBUF (feature-major, heads in half-partitions)
            qfullT = bigs.tile([P, CH, TOK], F32R, tag="qg8")

            def q_consume(m, psum):
                nc.vector.tensor_copy(out=qfullT[:, m, :], in_=psum[:])

            if "qkv" in stages:
                for g0 in range(CH):
                    qkv_mgroup(range(g0, g0 + 1), 0, q_consume)
            v_views = tuple(
                kv_outs[i][:].rearrange(
                    "(r u tc p) (hh d) -> r u tc p hh d",
                    r=RANKS,
                    u=2,
                    tc=TOK // P,
                    hh=H // 2,
                )[:, 1]
                for i in range(2)
            )
            def load_kp(hp):
                kp = kvpool.tile(
                    [P, RANKS, TOK // P, P], F32R, tag="kp", name=f"kp{hp}"
                )
                nc.gpsimd.dma_start(
                    kp[:],
                    kv_outs[hp // 4][:]
                    .rearrange("(r q p) t -> r q p t", r=RANKS, q=CH)[
                        :, hp % 4, :, :
                    ]
                    .rearrange("r p (tc tk) -> p r tc tk", tk=P),
                )
                return kp

            def load_vf(h):
                vf = kvpool.tile(
                    [P, KC, DH + 1], F32R, tag="vf", bufs=3, name=f"vf{h}"
                )
                vf4 = vf[:, :, 0:DH].rearrange("p (r tc) d -> p r tc d", r=RANKS)
                for r_ in range(RANKS):
                    nc.gpsimd.dma_start(
                        vf4[:, r_, :, :],
                        v_views[h // 8][r_, :, :, h % 8, :].rearrange(
                            "tc p d -> p tc d"
                        ),
                    )
                nc.vector.memset(vf[:, :, DH : DH + 1].bitcast(F32), 1.0)
                return vf

            pre_kp = load_kp(0) if "attn" in stages else None
            pre_vf = load_vf(0) if "attn" in stages else None
            if "qkv" in stages:
                v_phase(1)
            st2_pool_cm.__exit__(None, None, None)


            # ---- stage 3: attention (head pairs share a kpair tile) ----
            ctxT = bigs.tile([P, CH, TOK], F32R, tag="ctxacc")

            attn_pools = (
                tc.tile_pool(name="ps_s", bufs=2, space="PSUM"),
                tc.tile_pool(name="ps_ctx", bufs=2, space="PSUM"),
            )
            ps_spool = attn_pools[0].__enter__()
            ps_ctx = attn_pools[1].__enter__()
            for hp in range(H // 2) if "attn" in stages else ():
                # K^T for heads (2hp, 2hp+1): 128 consecutive rows per rank
                kp = pre_kp if hp == 0 else load_kp(hp)
                for h in (2 * hp, 2 * hp + 1):
                    half = slice((h % 2) * 64, (h % 2) * 64 + 64)
                    vf = pre_vf if h == 0 else load_vf(h)
                    psum_c = ps_ctx.tile([DH + 1, TOK], F32, tag="ctx")
                    kc0 = 0
                    for nb in (3, 3, 3, 3, 2, 2):
                        ps_s = ps_spool.tile([P, 3 * TOK], F32, tag="s")
                        for j in range(nb):
                            kc = kc0 + j
                            r, tcc = divmod(kc, TOK // P)
                            nc.tensor.matmul(
                                ps_s[:, j * TOK : (j + 1) * TOK],
                                kp[half, r, tcc, :],
                                qfullT[half, h // 2, :],
                                start=True,
                                stop=True,
                            )
                        pt = ppool.tile([P, 3 * TOK], F32R, tag="p")
                        if "exp" in stages:
                            nc.scalar.activation(
                                out=pt[:, : nb * TOK],
                                in_=ps_s[:, : nb * TOK],
                                func=AF.Exp,
                                scale=SCALE,
                            )
                        for j in range(nb):
                            kc = kc0 + j
                            nc.tensor.matmul(
                                psum_c[:],
                                vf[:, kc, :],
                                pt[:, j * TOK : (j + 1) * TOK],
                                start=(kc == 0),
                                stop=(kc == KC - 1),
                            )
                        kc0 += nb
                    rrow = rows.tile([1, TOK], F32, tag="r")
                    nc.vector.reciprocal(rrow[:], psum_c[DH : DH + 1, :])
                    rb = bcpool.tile([64, TOK], F32, tag="rb", bufs=3)
                    nc.gpsimd.partition_broadcast(rb[:], rrow[:])
                    nc.vector.tensor_tensor(
                        ctxT[half, h // 2, :], psum_c[0:DH, :], rb[:], OP.mult
                    )

            attn_pools[1].__exit__(None, None, None)
            attn_pools[0].__exit__(None, None, None)

            # ---- stage 4: proj + residual (ch-outer over m-groups) ----
            st4_pool_cm = tc.tile_pool(name="ps_mm4", bufs=5, space="PSUM")
            ps_mlp = st4_pool_cm.__enter__()
            x2 = bigs.tile([P, CH, TOK], F32, tag="x2")

            def proj_group(ms):
                ws, psums = [], []
                for m in ms:
                    w = wpool.tile([P, CH, P], F32R, tag="wcol8")
                    nc.sync.dma_start(
                        w[:],
                        proj_wT[:, m * P : (m + 1) * P].rearrange(
                            "(ch p) o -> p ch o", p=P
                        ),
                    )
                    ws.append(w)
                    psums.append(ps_mlp.tile([P, TOK], F32, tag="acc", name=f"ps_{m}"))
                for ch in range(CH):
                    for i in range(len(ms)):
                        nc.tensor.matmul(
                            psums[i][:],
                            ws[i][:, ch, :],
                            ctxT[:, ch, :],
                            start=(ch == 0),
                            stop=(ch == CH - 1),
                        )
                for i, m in enumerate(ms):
                    attn_sb = stg.tile([P, TOK], F32, tag="stg", bufs=2)
                    nc.scalar.activation(
                        out=attn_sb[:],
                        in_=psums[i][:],
                        func=AF.Identity,
                        bias=projb_sb[:, m : m + 1],
                    )
                    xc = work.tile([P, TOK], F32, tag="xc")
                    nc.sync.dma_start(xc[:], xT_chunks[:, m, :])
                    nc.vector.tensor_add(out=x2[:, m, :], in0=attn_sb[:], in1=xc[:])

            if "proj" in stages:
                for g0 in range(CH):
                    proj_group(range(g0, g0 + 1))

            # ---- stage 5: LN2 stats only (affine folded into fc1 weights) ----
            x2r = bigs.tile([P, CH, TOK], F32R, tag="h12")
            rstd2_b = c2_b = None
            if "ln2" in stages:
                with tc.tile_pool(name="ps_row2", bufs=2, space="PSUM") as prow:
                    rstd2_b, c2_b = layer_norm(
                        lambda ch: x2[:, ch, :], ln2g_sb, ln2b_sb, None, prow,
                        xr_dst=x2r,
                    )

            # ---- stage 6: MLP in hidden-quarters with SBUF accumulator ----
            acc_sb = bigs.tile([P, CH, TOK], F32, tag="ctxacc")
            QH = 8  # hidden chunks per quarter
            for q in range(HCH // QH) if "mlp" in stages else ():
                g8 = bigs.tile([P, QH, TOK], F32R, tag="qg8")
                for mg in range(QH):
                    ws, psums = [], []
                    for i in range(1):
                        m = q * QH + mg + i
                        w = wpool.tile([P, CH, P], F32R, tag="wcol8")
                        nc.gpsimd.dma_start(
                            w[:],
                            fc1_wT[:, m * P : (m + 1) * P].rearrange(
                                "(ch p) o -> p ch o", p=P
                            ),
                        )
                        ws.append(w)
                        psums.append(ps_mlp.tile([P, TOK], F32, tag="acc", name=f"ps_{m}"))
                    for ch in range(CH):
                        for i in range(1):
                            nc.tensor.matmul(
                                psums[i][:],
                                ws[i][:, ch, :],
                                x2r[:, ch, :],
                                start=(ch == 0),
                                stop=(ch == CH - 1),
                            )
                    for i in range(1):
                        m = q * QH + mg + i
                        tmp = work.tile([P, TOK], F32, tag="t1", name=f"cor_{m}")
                        nc.vector.tensor_scalar(
                            out=tmp[:],
                            in0=c2_b[:],
                            scalar1=fc1mg_sb[:, m : m + 1],
                            scalar2=None,
                            op0=OP.mult,
                        )
                        t1 = work.tile([P, TOK], F32, tag="t1", name=f"t1_{m}")
                        nc.vector.tensor_tensor(
                            t1[:], psums[i][:], rstd2_b[:], OP.mult
                        )
                        nc.vector.tensor_add(t1[:], t1[:], tmp[:])
                        nc.scalar.activation(
                            out=g8[:, mg + i, :],
                            in_=t1[:],
                            func=AF.Gelu,
                            bias=fc1b_sb[:, m : m + 1],
                        )
                for m2g in range(CH):
                    ws2, psums2 = [], []
                    for i in range(1):
                        m2 = m2g + i
                        w2 = wpool.tile([P, QH, P], F32R, tag="w8", bufs=3)
                        nc.gpsimd.dma_start(
                            w2[:],
                            fc2_wT[
                                q * QH * P : (q + 1) * QH * P, m2 * P : (m2 + 1) * P
                            ].rearrange("(hc p) o -> p hc o", p=P),
                        )
                        ws2.append(w2)
                        psums2.append(ps_mlp.tile([P, TOK], F32, tag="acc", name=f"ps2_{m2}"))
                    for hc in range(QH):
                        for i in range(1):
                            nc.tensor.matmul(
                                psums2[i][:],
                                ws2[i][:, hc, :],
                                g8[:, hc, :],
                                start=(hc == 0),
                                stop=(hc == QH - 1),
                            )
                    for i in range(1):
                        m2 = m2g + i
                        if q == 0:
                            nc.vector.tensor_copy(
                                out=acc_sb[:, m2, :], in_=psums2[i][:]
                            )
                        elif q < HCH // QH - 1:
                            nc.vector.tensor_add(
                                out=acc_sb[:, m2, :],
                                in0=acc_sb[:, m2, :],
                                in1=psums2[i][:],
                            )
                        else:
                            o_sb = stg.tile([P, TOK], F32, tag="stg", bufs=2)
                            nc.scalar.activation(
                                out=o_sb[:],
                                in_=psums2[i][:],
                                func=AF.Identity,
                                bias=fc2b_sb[:, m2 : m2 + 1],
                            )
                            nc.vector.tensor_add(
                                out=o_sb[:], in0=o_sb[:], in1=acc_sb[:, m2, :]
                            )
                            o_f = stg.tile([P, TOK], F32, tag="of", bufs=2)
                            nc.vector.tensor_add(
                                out=o_f[:], in0=o_sb[:], in1=x2[:, m2, :]
                            )
                            nc.sync.dma_start(
                                outT[m2 * P : (m2 + 1) * P, :], o_f[:]
                            )
            if "mlp" not in stages:
                o_f = stg.tile([P, TOK], F32, tag="of", bufs=2)
                nc.vector.tensor_copy(out=o_f[:], in_=x2[:, 0, :])
                nc.sync.dma_start(outT[0:P, :], o_f[:])
            st4_pool_cm.__exit__(None, None, None)

    if do_compile:
        nc.compile()
    return nc


def build_program_ablated(stages):
    return build_program(stages=stages, do_compile=False)


_CACHE = {}


def _get_program():
    if "nc" not in _CACHE:
        _CACHE["nc"] = build_program()
    return _CACHE["nc"]


def _prep_inputs(inputs):
    """Host-side sharding + layout prep. Returns per-core in_maps."""
    x = np.asarray(inputs["x"], dtype=np.float32)
    shared = {
        "qkv_wT": round_fp32r(np.asarray(inputs["qkv_w"], np.float32).T),
        "proj_wT": round_fp32r(np.asarray(inputs["proj_w"], np.float32).T),
        "fc1_wT": round_fp32r(
            (
                np.asarray(inputs["fc1_w"], np.float32)
                * np.asarray(inputs["ln2_g"], np.float32)[None, :]
            ).T
        ),
        "fc2_wT": round_fp32r(np.asarray(inputs["fc2_w"], np.float32).T),
        "ln1g": _ln_stripe(inputs["ln1_g"]),
        "ln1b": _ln_stripe(inputs["ln1_b"]),
        "ln2g": _ln_stripe(inputs["ln2_g"]),
        "ln2b": _ln_stripe(inputs["ln2_b"]),
        "projb": _ln_stripe(inputs["proj_b"]),
        "fc1b": _ln_stripe(
            np.asarray(inputs["fc1_b"], np.float32)
            + np.asarray(inputs["fc1_w"], np.float32)
            @ np.asarray(inputs["ln2_b"], np.float32)
        ),
        "fc1mg": _ln_stripe(
            -(
                np.asarray(inputs["fc1_w"], np.float32)
                @ np.asarray(inputs["ln2_g"], np.float32)
            )
        ),
        "fc2b": _ln_stripe(inputs["fc2_b"]),
    }
    in_maps = []
    for c in range(NCORES):
        b, blk = divmod(c, RANKS)
        xblk = x[b, blk * TOK : (blk + 1) * TOK, :]  # [TOK, D]
        m = dict(shared)
        m["xT"] = np.ascontiguousarray(xblk.T)  # [D, TOK]
        in_maps.append(m)
    return in_maps


def _assemble(results):
    out = np.empty((B, N, D), dtype=np.float32)
    for c in range(NCORES):
        b, blk = divmod(c, RANKS)
        out[b, blk * TOK : (blk + 1) * TOK, :] = results[c]["outT"].T
    return out


def run_device(inputs, **kwargs):
    nc = _get_program()
    in_maps = _prep_inputs(inputs)
    res = run_bass_kernel_spmd(nc, in_maps, core_ids=list(range(NCORES)), **kwargs)
    return _assemble(res.results), res


def kernel(**inputs) -> np.ndarray:
    out, _ = run_device(inputs)
    return out



# revision 6
# speedup vs baseline: 1.1369x; 1.1369x over previous
"""Trainium2 Bass kernel v2: fp8 DoubleRow attention + bf16 MLP.

Sharding: sequence-parallel over 8 cores (512 tokens each, batch = core//4).
One 4-rank AllGather carries fp8 K (feature-major) + fp8 V (token-major).

Precision plan (validated in numpy, max_rel ~1.0e-2 vs 2e-2 gate):
  - weights qkv/proj: e4m3 x32 host-scaled; fc1/fc2: bf16 (MLP dominates error)
  - h1/q/k/v/pt/ctx: e4m3 (q,k,v at sigma~16 via 0.5 consume scale)
  - softmax exp: constant shift C=4 (cancels in normalize); split between
    Act (native Exp -> fp8) and DVE (Schraudolph: psum*a+b -> uint8 whose
    bit pattern IS e4m3 2^x; floor-vs-round ambiguity is a constant factor
    that cancels in the softmax normalize)
  - x residual fp32r; LN stats via ones-matmul (fp8 DoubleRow for LN1 on
    host-provided x8, fp32r for LN2)
DoubleRow pair slots: chunk pairs for QKV/ctx/proj; (k, zeros) for scores
(d=64 contraction cannot pair; zero slot makes the 0.5 cyc/row rate legal).
"""
import sys

sys.path.insert(0, "/opt/trn_rl_repo")
import numpy as np
import ml_dtypes
import concourse.bass as bass
import concourse.mybir as mybir
import concourse.tile as tile
from concourse import bacc
from concourse.bass_utils import run_bass_kernel_spmd

B, N, D = 2, 2048, 1024
H, DH = 16, 64
HID = 4096
NCORES = 8
TOK = (B * N) // NCORES  # 512
EPS = 1e-5
SCALE = DH**-0.5
P = 128
CH = D // P  # 8
KC = N // P  # 16
HCH = HID // P  # 32
RANKS = 4
CSH = 4.0  # exp arg shift, cancels in softmax
LN2_ = float(np.log(2.0))
# Schraudolph uint8-as-e4m3: y = psum * SA + SB
SA = 8.0 * (2.0**-11) / LN2_
SB = 56.5 - 8.0 * CSH / LN2_

F32 = mybir.dt.float32
F32R = mybir.dt.float32r
F8 = mybir.dt.float8e4
BF = mybir.dt.bfloat16
U8 = mybir.dt.uint8
AF = mybir.ActivationFunctionType
OP = mybir.AluOpType
DR = mybir.MatmulPerfMode.DoubleRow

REPLICA_GROUPS = [[0, 1, 2, 3], [4, 5, 6, 7]]

KV_K = D * TOK  # bytes of K region (fp8 feature-major [1024, 512])
DHP = DH + 16  # per-head stride in V region: 64 v + 1 ones + 15 pad
# (dual-fp8 LdWeights requires 16B-aligned weight base addresses)
KV_V = TOK * (H * DHP)  # V region [512, 1280]
KV_SZ = KV_K + KV_V


def round_fp32r(x: np.ndarray) -> np.ndarray:
    u = np.ascontiguousarray(x, dtype=np.float32).view(np.uint32)
    u = (u + 0x7FF + ((u >> 12) & 1)) & np.uint32(0xFFFFF000)
    return u.view(np.float32)


def _stripe(v: np.ndarray) -> np.ndarray:
    """[M] -> [P, M//P] with col m, part p = v[m*128+p]."""
    return np.ascontiguousarray(np.asarray(v, np.float32).reshape(-1, P).T)


def build_program(do_compile=True):
    nc = bacc.Bacc("TRN2", target_bir_lowering=False, debug=False, num_devices=NCORES)

    xT = nc.dram_tensor("xT", [D, TOK], F32, kind="ExternalInput").ap()
    x8T = nc.dram_tensor("x8T", [D, TOK], F8, kind="ExternalInput").ap()
    # weight tiles, DMA-contiguous per partition
    wqk8 = nc.dram_tensor("wqk8", [16, P, CH * P], F8, kind="ExternalInput").ap()
    wv8 = nc.dram_tensor("wv8", [2, P, CH * TOK], F8, kind="ExternalInput").ap()
    wp8 = nc.dram_tensor("wp8", [CH, P, CH * P], F8, kind="ExternalInput").ap()
    w1bf = nc.dram_tensor("w1bf", [CH, P, 4 * CH * P], BF, kind="ExternalInput").ap()
    w2bf = nc.dram_tensor("w2bf", [CH, P, HCH * P], BF, kind="ExternalInput").ap()
    biasqk = nc.dram_tensor("biasqk", [P, 16], F32, kind="ExternalInput").ap()
    biaspf = nc.dram_tensor("biaspf", [P, 16], F32, kind="ExternalInput").ap()
    fc1b = nc.dram_tensor("fc1b", [P, HCH], F32, kind="ExternalInput").ap()
    outT = nc.dram_tensor("outT", [D, TOK], F32, kind="ExternalOutput").ap()

    with tile.TileContext(nc) as tc:
        with (
            tc.tile_pool(name="consts", bufs=1) as consts,
            tc.tile_pool(name="bigs", bufs=1) as bigs,
            tc.tile_pool(name="work", bufs=3) as work,
            tc.tile_pool(name="wpool", bufs=2) as wpool,
            tc.tile_pool(name="kvz", bufs=2) as kvz,
            tc.tile_pool(name="pp", bufs=2) as ppool,
            tc.tile_pool(name="rows", bufs=3) as rows,
            tc.tile_pool(name="bc", bufs=2) as bcpool,
            tc.tile_pool(name="stg", bufs=3) as stg,
            tc.tile_pool(name="dram", bufs=1, space="DRAM") as dram,
        ):
            ones8 = consts.tile([P, 1], F8)
            nc.vector.memset(ones8[:].bitcast(U8), 0x38)  # e4m3 1.0
            ones_r = consts.tile([P, 1], F32R)
            nc.vector.memset(ones_r[:].bitcast(F32), 1.0)
            eps_row = consts.tile([1, 1], F32, tag="eps")
            nc.vector.memset(eps_row[:], EPS)
            negc_row = consts.tile([P, 1], F32, tag="negc")
            nc.vector.memset(negc_row[:], -CSH)
            onesv = consts.tile([P, 4, H], F8, tag="onesv")
            nc.vector.memset(onesv[:].bitcast(U8), 0x38)
            biasqk_sb = consts.tile([P, 16], F32, tag="bqk")
            biaspf_sb = consts.tile([P, 16], F32, tag="bpf")
            fc1b_sb = consts.tile([P, HCH], F32, tag="b1")
            nc.sync.dma_start(biasqk_sb[:], biasqk[:])
            nc.sync.dma_start(biaspf_sb[:], biaspf[:])
            nc.sync.dma_start(fc1b_sb[:], fc1b[:])

            kv_in = dram.tile([KV_SZ], F8, tag="kvin")
            kv_out = dram.tile([RANKS * KV_SZ], F8, tag="kvout")
            vk_in = kv_in[0:KV_K].rearrange("(f t) -> f t", t=TOK)
            vv_in = kv_in[KV_K:KV_SZ].rearrange(
                "(t v) -> t v", v=H * DHP
            )
            kv_or = kv_out[:].rearrange("(r x) -> r x", r=RANKS)

            # ---- LN1 ---- (x8 first: stats depend on it; xr arrives later)
            x8_sb = bigs.tile([P, CH, TOK], F8, tag="x8")
            nc.sync.dma_start(
                x8_sb[:], x8T.rearrange("(ch p) t -> p ch t", p=P)
            )
            xr_sb = bigs.tile([P, CH, TOK], F32, tag="xr")
            nc.sync.dma_start(
                xr_sb[:], xT.rearrange("(ch p) t -> p ch t", p=P)
            )
            sq8 = bigs.tile([P, CH, TOK], F8, tag="h1")  # released before h1

            def ln_rows(psum_mu, psum_s2, name):
                mu = rows.tile([1, TOK], F32, tag="r", name=f"mu{name}")
                nc.vector.tensor_scalar_mul(mu[:], psum_mu[:], 1.0 / D)
                var = rows.tile([1, TOK], F32, tag="r", name=f"va{name}")
                nc.vector.tensor_tensor(var[:], mu[:], mu[:], OP.mult)
                ex2 = rows.tile([1, TOK], F32, tag="r", name=f"e2{name}")
                nc.vector.tensor_scalar_mul(ex2[:], psum_s2[:], 1.0 / D)
                nc.vector.tensor_sub(var[:], ex2[:], var[:])
                rstd = rows.tile([1, TOK], F32, tag="r", name=f"rs{name}")
                nc.scalar.activation(
                    out=rstd[:], in_=var[:], func=AF.Sqrt, bias=eps_row[:]
                )
                nc.vector.reciprocal(rstd[:], rstd[:])
                cpos = rows.tile([1, TOK], F32, tag="r", name=f"cp{name}")
                nc.vector.tensor_tensor(cpos[:], mu[:], rstd[:], OP.mult)
                rstd_b = bcpool.tile([P, TOK], F32, tag="bc", name=f"rb{name}")
                nc.gpsimd.partition_broadcast(rstd_b[:], rstd[:])
                c_b = bcpool.tile([P, TOK], F32, tag="bc", name=f"cb{name}")
                nc.gpsimd.partition_broadcast(c_b[:], cpos[:])
                return rstd_b, c_b

            with tc.tile_pool(name="ps_row1", bufs=2, space="PSUM") as prow:
                psum_mu = prow.tile([1, TOK], F32, tag="row")
                psum_s2 = prow.tile([1, TOK], F32, tag="row")
                for ch in range(CH):
                    eng = nc.vector if ch % 2 == 0 else nc.gpsimd
                    eng.tensor_tensor(
                        sq8[:, ch, :], x8_sb[:, ch, :], x8_sb[:, ch, :], OP.mult
                    )
                for ch in range(CH):
                    nc.tensor.matmul(
                        psum_mu[:],
                        ones8[:],
                        x8_sb[:, ch, :],
                        start=(ch == 0),
                        stop=(ch == CH - 1),
                    )
                for ch in range(CH):
                    nc.tensor.matmul(
                        psum_s2[:],
                        ones8[:],
                        sq8[:, ch, :],
                        start=(ch == 0),
                        stop=(ch == CH - 1),
                    )
                rstd1_b, c1_b = ln_rows(psum_mu, psum_s2, "1")

            h1 = bigs.tile([P, CH, TOK], F8, tag="h1")
            for ch in range(CH):
                eng = nc.vector if ch % 2 == 0 else nc.gpsimd
                t1 = work.tile([P, TOK], F32, tag="t1")
                eng.tensor_tensor(t1[:], xr_sb[:, ch, :], rstd1_b[:], OP.mult)
                eng.tensor_tensor(h1[:, ch, :], t1[:], c1_b[:], OP.subtract)

            # ---- QKV ----
            st2 = tc.tile_pool(name="ps_mm2", bufs=5, space="PSUM")
            ps_acc = st2.__enter__()

            def qkv_block(wt, i, m, consume):
                psum = ps_acc.tile([P, TOK], F32, tag="acc", name=f"ps_{m}")
                for j in range(CH // 2):
                    nc.tensor.matmul(
                        psum[:],
                        wt[:, i, 2 * j : 2 * j + 2, :],
                        h1[:, 2 * j : 2 * j + 2, :],
                        start=(j == 0),
                        stop=(j == CH // 2 - 1),
                        perf_mode=DR,
                    )
                consume(m, psum)

            def k_consume(m, psum):
                k8 = stg.tile([P, TOK], F8, tag="cp", name=f"k8_{m}")
                nc.scalar.activation(
                    out=k8[:],
                    in_=psum[:],
                    func=AF.Identity,
                    bias=biasqk_sb[:, 8 + m : 9 + m],
                    scale=0.5,
                )
                nc.sync.dma_start(vk_in[m * P : (m + 1) * P, :], k8[:])

            # K blocks (g1 folded into weights on host; cols D..2D of qkv_w)
            wk_t = []
            for g in range(2):
                wt = wpool.tile([P, 4, CH, P], F8, tag="wq", name=f"wk{g}")
                nc.sync.dma_start(
                    wt[:],
                    wqk8[8 + 4 * g : 12 + 4 * g].rearrange("i p (ch o) -> p i ch o", ch=CH),
                )
                wk_t.append(wt)
            for m in range(CH):
                qkv_block(wk_t[m // 4], m % 4, m, k_consume)

            # V (token-major): lhsT = h1 chunk-pair, rhs = wv columns
            for ph in range(2):
                wv_t = wpool.tile([P, CH, TOK], F8, tag="wq", name=f"wv{ph}")
                nc.sync.dma_start(
                    wv_t[:], wv8[ph].rearrange("p (ch v) -> p ch v", ch=CH)
                )
                for tt_ in range(TOK // P):
                    psum = ps_acc.tile([P, TOK], F32, tag="acc", name=f"pv{ph}_{tt_}")
                    for j in range(CH // 2):
                        nc.tensor.matmul(
                            psum[:],
                            h1[:, 2 * j : 2 * j + 2, tt_ * P : (tt_ + 1) * P],
                            wv_t[:, 2 * j : 2 * j + 2, :],
                            start=(j == 0),
                            stop=(j == CH // 2 - 1),
                            perf_mode=DR,
                        )
                    v8 = stg.tile([P, TOK], F8, tag="cp", name=f"v8_{ph}_{tt_}")
                    nc.scalar.activation(
                        out=v8[:], in_=psum[:], func=AF.Identity, scale=0.5
                    )
                    dst = vv_in[
                        tt_ * P : (tt_ + 1) * P,
                        ph * 8 * DHP : (ph + 1) * 8 * DHP,
                    ].rearrange("t (h c) -> t h c", c=DHP)[:, :, 0:DH]
                    nc.sync.dma_start(
                        dst, v8[:].rearrange("t (h d) -> t h d", d=DH)
                    )

            for tc_ in range(RANKS):
                ones_dst = vv_in[tc_ * P : (tc_ + 1) * P, :].rearrange(
                    "p (h c) -> p h c", c=DHP
                )[:, :, DH : DH + 1]
                nc.sync.dma_start(
                    ones_dst, onesv[:, tc_, :].rearrange("p (h c) -> p h c", c=1)
                )
            nc.gpsimd.collective_compute(
                "AllGather",
                OP.bypass,
                ins=[kv_in[:].opt()],
                outs=[kv_out[:].opt()],
                replica_groups=REPLICA_GROUPS,
            )

            # Q blocks -> SBUF (chunk 8 duplicates chunk 7 for the hp=7 rhs pair)
            qT = bigs.tile([P, CH + 1, TOK], F8, tag="qT")

            def q_consume(m, psum):
                nc.vector.tensor_scalar(
                    out=qT[:, m, :],
                    in0=psum[:],
                    scalar1=0.5,
                    scalar2=biasqk_sb[:, m : m + 1],
                    op0=OP.mult,
                    op1=OP.add,
                )

            for g in range(2):
                wt = wpool.tile([P, 4, CH, P], F8, tag="wq", name=f"wq{g}")
                nc.sync.dma_start(
                    wt[:],
                    wqk8[4 * g : 4 * g + 4].rearrange("i p (ch o) -> p i ch o", ch=CH),
                )
                for i in range(4):
                    qkv_block(wt, i, 4 * g + i, q_consume)
            nc.sync.dma_start(qT[:, CH, :], qT[:, CH - 1, :])
            st2.__exit__(None, None, None)

            # ---- attention ----
            # kp: [P(2 heads' d), 2, KC, P] fp8; slot0 = K data, slot1 = zeros
            kp_t = []
            for i in range(2):
                t = kvz.tile([P, 2, KC, P], F8, tag="kp", name=f"kp{i}")
                nc.vector.memset(t[:, 1, :, :].bitcast(U8), 0)
                kp_t.append(t)
            # vfull: [P(key-in-chunk), KC, 16*(64+1)] fp8, ones pre-gathered
            vfull = bigs.tile([P, KC, H * DHP], F8, tag="vfull")
            for r in range(RANKS):
                src = (
                    kv_or[r : r + 1, KV_K:KV_SZ]
                    .rearrange("o (t v) -> o t v", v=H * DHP)[0]
                    .rearrange("(tc p) v -> p tc v", p=P)
                )
                nc.sync.dma_start(vfull[:, 4 * r : 4 * r + 4, :], src)

            def load_kp(hp, t):
                src = (
                    kv_or[:, 0:KV_K]
                    .rearrange("r (f t) -> r f t", t=TOK)[
                        :, hp * P : (hp + 1) * P, :
                    ]
                    .rearrange("r p t -> p r t")
                )
                nc.sync.dma_start(t[:, 0, :, :], src)

            ctxT = bigs.tile([P, CH, TOK], F8, tag="x8")  # reuse x8 region
            groups = [(0, 3), (3, 3), (6, 3), (9, 3), (12, 2), (14, 2)]
            pairs_after = {1: [0, 1, 2], 2: [3], 3: [4, 5], 4: [6], 5: [7]}

            att_pools = (
                tc.tile_pool(name="ps_s", bufs=2, space="PSUM"),
                tc.tile_pool(name="ps_ctx", bufs=2, space="PSUM"),
            )
            ps_spool = att_pools[0].__enter__()
            ps_ctx = att_pools[1].__enter__()
            eidx = 0
            for hp in range(H // 2):
                kp = kp_t[hp % 2]
                load_kp(hp, kp)
                for hh in range(2):
                    h = 2 * hp + hh
                    half = slice(hh * DH, hh * DH + DH)
                    qpair = qT[half, hp : hp + 2, :]
                    pt = ppool.tile([P, KC, TOK], F8, tag="pt", name=f"pt{h}")
                    psum_c = ps_ctx.tile([DH + 1, TOK], F32, tag="ctx")
                    for gi, (kc0, nb) in enumerate(groups):
                        ps_s = ps_spool.tile([P, 3 * TOK], F32, tag="s")
                        for j in range(nb):
                            nc.tensor.matmul(
                                ps_s[:, j * TOK : (j + 1) * TOK],
                                kp[half, :, kc0 + j, :],
                                qpair,
                                start=True,
                                stop=True,
                                perf_mode=DR,
                            )
                        dst = pt[:, kc0 : kc0 + nb, :]
                        # alternate Act/DVE so consecutive groups of a head
                        # never serialize on one engine (GPSIMD can't read
                        # PSUM per the BIR verifier)
                        if (gi + h) % 2 == 0:
                            nc.scalar.activation(
                                out=dst,
                                in_=ps_s[:, : nb * TOK],
                                func=AF.Exp,
                                bias=negc_row[:],
                                scale=2.0**-11,
                            )
                        else:
                            nc.vector.tensor_scalar(
                                out=dst.bitcast(U8),
                                in0=ps_s[:, : nb * TOK],
                                scalar1=SA,
                                scalar2=SB,
                                op0=OP.mult,
                                op1=OP.add,
                            )
                        eidx += 1
                        for pj in pairs_after.get(gi, []):
                            nc.tensor.matmul(
                                psum_c[:],
                                vfull[
                                    :,
                                    2 * pj : 2 * pj + 2,
                                    h * DHP : h * DHP + DH + 1,
                                ],
                                pt[:, 2 * pj : 2 * pj + 2, :],
                                start=(pj == 0),
                                stop=(pj == KC // 2 - 1),
                                perf_mode=DR,
                            )
                    rrow = rows.tile([1, TOK], F32, tag="r", name=f"rr{h}")
                    nc.vector.reciprocal(rrow[:], psum_c[DH : DH + 1, :])
                    rb = bcpool.tile([DH, TOK], F32, tag="rb", name=f"rb{h}")
                    nc.gpsimd.partition_broadcast(rb[:], rrow[:])
                    nc.vector.tensor_tensor(
                        ctxT[half, hp, :], psum_c[0:DH, :], rb[:], OP.mult
                    )
            att_pools[1].__exit__(None, None, None)
            att_pools[0].__exit__(None, None, None)

            # ---- proj + residual ----
            st4 = tc.tile_pool(name="ps_mm4", bufs=5, space="PSUM")
            ps_mlp = st4.__enter__()
            x2 = bigs.tile([P, CH, TOK], F32R, tag="x2")
            for g in range(2):
                wt = wpool.tile([P, 4, CH, P], F8, tag="wq", name=f"wpj{g}")
                nc.sync.dma_start(
                    wt[:],
                    wp8[4 * g : 4 * g + 4].rearrange("i p (ch o) -> p i ch o", ch=CH),
                )
                for i in range(4):
                    m = 4 * g + i
                    psum = ps_mlp.tile([P, TOK], F32, tag="acc", name=f"pp_{m}")
                    for j in range(CH // 2):
                        nc.tensor.matmul(
                            psum[:],
                            wt[:, i, 2 * j : 2 * j + 2, :],
                            ctxT[:, 2 * j : 2 * j + 2, :],
                            start=(j == 0),
                            stop=(j == CH // 2 - 1),
                            perf_mode=DR,
                        )
                    attn_sb = stg.tile([P, TOK], F32, tag="stg", name=f"at_{m}")
                    nc.scalar.activation(
                        out=attn_sb[:],
                        in_=psum[:],
                        func=AF.Identity,
                        bias=biaspf_sb[:, m : m + 1],
                        scale=2.0**-9,
                    )
                    nc.gpsimd.tensor_tensor(
                        x2[:, m, :], attn_sb[:], xr_sb[:, m, :], OP.add
                    )  # f32r out: rounded on write for the LN2 stats matmul

            # ---- LN2 (fp32r stats on x2) ----
            with tc.tile_pool(name="ps_row2", bufs=2, space="PSUM") as prow:
                psum_mu2 = prow.tile([1, TOK], F32, tag="row")
                psum_s22 = prow.tile([1, TOK], F32, tag="row")
                for ch in range(CH):
                    nc.tensor.matmul(
                        psum_mu2[:],
                        ones_r[:],
                        x2[:, ch, :],
                        start=(ch == 0),
                        stop=(ch == CH - 1),
                    )
                    sq = work.tile([P, TOK], F32R, tag="t1", name=f"sq2_{ch}")
                    nc.gpsimd.tensor_tensor(
                        sq[:],
                        x2[:, ch, :].bitcast(F32),
                        x2[:, ch, :].bitcast(F32),
                        OP.mult,
                    )
                    nc.tensor.matmul(
                        psum_s22[:],
                        ones_r[:],
                        sq[:],
                        start=(ch == 0),
                        stop=(ch == CH - 1),
                    )
                rstd2_b, c2_b = ln_rows(psum_mu2, psum_s22, "2")

            xn = bigs.tile([P, CH, TOK], BF, tag="xn")
            for ch in range(CH):
                t1 = work.tile([P, TOK], F32, tag="t1", name=f"t2_{ch}")
                nc.gpsimd.tensor_tensor(
                    t1[:], x2[:, ch, :].bitcast(F32), rstd2_b[:], OP.mult
                )
                nc.vector.tensor_tensor(xn[:, ch, :], t1[:], c2_b[:], OP.subtract)

            # ---- MLP (bf16) ----
            gbf = bigs.tile([P, HCH, TOK], BF, tag="g")
            for g in range(CH):
                w1 = wpool.tile([P, 4, CH, P], BF, tag="w1", name=f"w1_{g}")
                nc.sync.dma_start(
                    w1[:], w1bf[g].rearrange("p (i ch o) -> p i ch o", i=4, ch=CH)
                )
                for i in range(4):
                    m = 4 * g + i
                    psum = ps_mlp.tile([P, TOK], F32, tag="acc", name=f"p1_{m}")
                    for ch in range(CH):
                        nc.tensor.matmul(
                            psum[:],
                            w1[:, i, ch, :],
                            xn[:, ch, :],
                            start=(ch == 0),
                            stop=(ch == CH - 1),
                        )
                    nc.scalar.activation(
                        out=gbf[:, m, :],
                        in_=psum[:],
                        func=AF.Gelu,
                        bias=fc1b_sb[:, m : m + 1],
                    )
            for m2 in range(CH):
                w2 = wpool.tile([P, HCH, P], BF, tag="w2", name=f"w2_{m2}")
                nc.sync.dma_start(
                    w2[:], w2bf[m2].rearrange("p (hc o) -> p hc o", hc=HCH)
                )
                psum = ps_mlp.tile([P, TOK], F32, tag="acc", name=f"p2_{m2}")
                for hc in range(HCH):
                    nc.tensor.matmul(
                        psum[:],
                        w2[:, hc, :],
                        gbf[:, hc, :],
                        start=(hc == 0),
                        stop=(hc == HCH - 1),
                    )
                o_sb = stg.tile([P, TOK], F32, tag="stg", name=f"o_{m2}")
                nc.scalar.activation(
                    out=o_sb[:],
                    in_=psum[:],
                    func=AF.Identity,
                    bias=biaspf_sb[:, 8 + m2 : 9 + m2],
                )
                o_f = stg.tile([P, TOK], F32, tag="of", bufs=2, name=f"of_{m2}")
                nc.vector.tensor_add(
                    out=o_f[:], in0=o_sb[:], in1=x2[:, m2, :].bitcast(F32)
                )
                nc.sync.dma_start(outT[m2 * P : (m2 + 1) * P, :], o_f[:])
            st4.__exit__(None, None, None)

    if do_compile:
        nc.compile()
    return nc


_CACHE = {}


def _get_program():
    if "nc" not in _CACHE:
        _CACHE["nc"] = build_program()
    return _CACHE["nc"]


def _prep_inputs(inputs):
    E4 = ml_dtypes.float8_e4m3
    x = np.asarray(inputs["x"], np.float32)
    g1 = np.asarray(inputs["ln1_g"], np.float32)
    b1 = np.asarray(inputs["ln1_b"], np.float32)
    g2 = np.asarray(inputs["ln2_g"], np.float32)
    b2 = np.asarray(inputs["ln2_b"], np.float32)
    qkv_w = np.asarray(inputs["qkv_w"], np.float32) * g1[None, :]
    proj_w = np.asarray(inputs["proj_w"], np.float32)
    fc1_w = np.asarray(inputs["fc1_w"], np.float32) * g2[None, :]
    fc2_w = np.asarray(inputs["fc2_w"], np.float32)

    qkv_bias = np.asarray(inputs["qkv_w"], np.float32) @ b1  # [3D]
    assert np.abs(qkv_bias[2 * D :]).max() == 0.0, "nonzero ln1_b v-bias unsupported"

    def wtile8(w, blocks):
        """w [O, D] -> [nb, P, CH*P] fp8 with [m, p, ch*128+o] = 32*w[m*128+o, ch*128+p]."""
        out = np.empty((len(blocks), P, CH * P), E4)
        for bi, m in enumerate(blocks):
            blk = w[m * P : (m + 1) * P, :] * 32.0  # [o 128, c 1024]
            out[bi] = (
                blk.reshape(P, CH, P).transpose(2, 1, 0).reshape(P, CH * P)
            ).astype(E4)
        return out

    def wtile_bf(w, nb, batch):
        """w [O, D] -> [nb//batch, P, batch*CH*P] bf16 tiles."""
        out = np.empty((nb // batch, P, batch * (w.shape[1] // P) * P), ml_dtypes.bfloat16)
        chn = w.shape[1] // P
        for g in range(nb // batch):
            t = np.empty((P, batch, chn, P), np.float32)
            for i in range(batch):
                m = g * batch + i
                blk = w[m * P : (m + 1) * P, :]  # [o, c]
                t[:, i] = blk.reshape(P, chn, P).transpose(2, 1, 0)
            out[g] = t.reshape(P, -1).astype(ml_dtypes.bfloat16)
        return out

    # V weights token-major: [ph, p, ch*512+vc] = 32*qkv_w'[2D+ph*512+vc, ch*128+p]
    wv = np.empty((2, P, CH * TOK), E4)
    for ph in range(2):
        blk = qkv_w[2 * D + ph * TOK : 2 * D + (ph + 1) * TOK, :] * 32.0  # [vc, c]
        wv[ph] = blk.reshape(TOK, CH, P).transpose(2, 1, 0).reshape(P, CH * TOK).astype(E4)

    bqk = np.zeros((P, 16), np.float32)
    bqk[:, 0:8] = _stripe(16.0 * qkv_bias[0:D])
    bqk[:, 8:16] = _stripe(16.0 * qkv_bias[D : 2 * D])
    bpf = np.zeros((P, 16), np.float32)
    bpf[:, 0:8] = _stripe(inputs["proj_b"])
    bpf[:, 8:16] = _stripe(inputs["fc2_b"])

    shared = {
        "wqk8": wtile8(qkv_w, list(range(16))),
        "wv8": wv,
        "wp8": wtile8(proj_w, list(range(CH))),
        "w1bf": wtile_bf(fc1_w, HCH, 4),
        "w2bf": wtile_bf(fc2_w, CH, 1),
        "biasqk": bqk,
        "biaspf": bpf,
        "fc1b": _stripe(
            np.asarray(inputs["fc1_b"], np.float32)
            + np.asarray(inputs["fc1_w"], np.float32) @ b2
        ),
    }
    in_maps = []
    for c in range(NCORES):
        b, blk = divmod(c, RANKS)
        xblk = x[b, blk * TOK : (blk + 1) * TOK, :]  # [TOK, D]
        xt = round_fp32r(np.ascontiguousarray(xblk.T))
        m = dict(shared)
        m["xT"] = xt
        m["x8T"] = xt.astype(E4)
        in_maps.append(m)
    return in_maps


def _assemble(results):
    out = np.empty((B, N, D), dtype=np.float32)
    for c in range(NCORES):
        b, blk = divmod(c, RANKS)
        out[b, blk * TOK : (blk + 1) * TOK, :] = results[c]["outT"].T
    return out


def run_device(inputs, **kwargs):
    nc = _get_program()
    in_maps = _prep_inputs(inputs)
    res = run_bass_kernel_spmd(nc, in_maps, core_ids=list(range(NCORES)), **kwargs)
    return _assemble(res.results), res


def kernel(**inputs) -> np.ndarray:
    out, _ = run_device(inputs)
    return out
